# revision 5
# baseline (speedup 1.0000x reference)
"""Trainium2 Bass kernel for nn_Column (nms_detection).

Computation (matches the reference exactly):
  out[t,k]  = sum_chw rec_field[t,chw] * weight[k,chw]        (32x512 <- contract 100000)
  pot       = out * (out > 10) ; spike = (out > 10)
  nspk[k]   = sum_t spike ; first[k] = min(32 - nspk, 31)
  values[k] = pot[first[k], k] ; v = max_k(values * (nspk>0)) * 32
  total     = nspk*values + nspk*v
  coef      = top-16 nonzero mask of total (== sequential argmax-suppress set)
  result    = spike * coef[broadcast]                          (32x512 of 0.0/1.0)

Distribution: contraction dim (100000) sharded 8 ways (12500 rows/core, padded
to 12544 = 98*128).  Each core computes a partial (32,512) using bf16 hi/lo
splitting: inputs are pre-split on the host into bf16 high+low parts (exactly
the decomposition the HW fp32 matmul path uses internally, so precision is
identical to fp32) and the PE runs 2 moving passes per 128-chunk (w_hi, w_lo)
against a packed stationary [x_hi | x_lo] (128,64), accumulating a (64,512)
PSUM whose halves are folded after the loop.  Partials are combined with one
64KB AllReduce.  Every core then redundantly computes the tiny k-WTA
epilogue; core 0's output is returned.
"""

import numpy as np
import ml_dtypes

import concourse.bacc as bacc
import concourse.mybir as mybir
from concourse.tile import TileContext
from concourse.bass_utils import run_bass_kernel_spmd

T = 32               # timesteps
K = 512              # out_channels / features
CTOT = 100000        # in_channels * rf_size * length (1*50*2000)
NCORES = 8
SH = CTOT // NCORES  # 12500 contraction rows per core
NCH = 98             # 128-row contraction chunks per core
SHP = NCH * 128      # 12544 (zero padded)
GROUP = 7            # chunks per W DMA group  (7*1024*128*2B = 1.75 MiB)
NG = NCH // GROUP    # 14 groups
THRESH = 10.0
F32 = mybir.dt.float32
BF16 = mybir.dt.bfloat16
NPBF16 = ml_dtypes.bfloat16

_CACHE = {}


def _build_nc():
    nc = bacc.Bacc("TRN2", target_bir_lowering=False, debug=False, num_devices=NCORES)

    # x: per chunk c the stationary block [x_hi | x_lo] (128,64) bf16
    x_d = nc.dram_tensor("x", [128, NCH * 2 * T], BF16, kind="ExternalInput")
    # w: per chunk c [w_hi (128,512) | w_lo (128,512)] bf16
    w_d = nc.dram_tensor("w", [128, NCH * 2 * K], BF16, kind="ExternalInput")
    oc_d = nc.dram_tensor("onescol", [T, 1], F32, kind="ExternalInput")
    o32_d = nc.dram_tensor("ones32", [T, T], F32, kind="ExternalInput")
    or_d = nc.dram_tensor("onesrow", [1, T], F32, kind="ExternalInput")
    tp_d = nc.dram_tensor("tpos32", [T, 1], F32, kind="ExternalInput")
    out_d = nc.dram_tensor("out", [T, K], F32, kind="ExternalOutput")

    with TileContext(nc) as tc:
        with (
            tc.tile_pool(name="sb", bufs=1) as sb,
            tc.tile_pool(name="wp", bufs=3) as wp,
            tc.tile_pool(name="ps", bufs=1, space="PSUM") as ps,
            tc.tile_pool(name="dram", bufs=1, space="DRAM") as dr,
        ):
            # X + consts on the scalar HWDGE ring; W groups alternate between
            # the sync and vector rings so the weight stream starts at t~0
            # and can use two hardware queues.
            xsb = sb.tile([128, NCH * 2 * T], BF16)
            nc.scalar.dma_start(xsb[:], x_d[:])
            oc = sb.tile([T, 1], F32)
            nc.scalar.dma_start(oc[:], oc_d[:])
            o32 = sb.tile([T, T], F32)
            nc.scalar.dma_start(o32[:], o32_d[:])
            orr = sb.tile([1, T], F32)
            nc.scalar.dma_start(orr[:], or_d[:])
            tpos = sb.tile([T, 1], F32)
            nc.scalar.dma_start(tpos[:], tp_d[:])

            # ---- matmul: per chunk 2 bf16 passes (w_hi, w_lo) against the
            # packed stationary [x_hi | x_lo]; (64,512) PSUM accumulator ----
            accum = ps.tile([2 * T, K], F32)
            for g in range(NG):
                wt = wp.tile([128, GROUP * 2 * K], BF16, tag="wt")
                eng = nc.sync if (g % 2 == 0) else nc.scalar
                eng.dma_start(wt[:], w_d[:, g * GROUP * 2 * K:(g + 1) * GROUP * 2 * K])
                for c in range(GROUP):
                    cc = g * GROUP + c
                    xst = xsb[:, cc * 2 * T:(cc + 1) * 2 * T]
                    nc.tensor.matmul(
                        accum[:],
                        xst,
                        wt[:, (2 * c) * K:(2 * c + 1) * K],
                        start=(cc == 0),
                        stop=False,
                    )
                    nc.tensor.matmul(
                        accum[:],
                        xst,
                        wt[:, (2 * c + 1) * K:(2 * c + 2) * K],
                        start=False,
                        stop=(cc == NCH - 1),
                    )

            # ---- fold hi/lo halves: part = accum[0:32] + accum[32:64] ----
            # PSUM can only feed one input per DVE op and DMA can't read PSUM,
            # so: copy lo half to SBUF (same partitions), DMA it down to
            # partitions 0-31, then add against the PSUM hi half.
            lo64 = sb.tile([2 * T, K], F32)
            nc.vector.tensor_copy(lo64[T:2 * T, :], accum[T:2 * T, :])
            nc.sync.dma_start(lo64[0:T, :], lo64[T:2 * T, :])
            part = sb.tile([T, K], F32)
            nc.vector.tensor_tensor(part[:], accum[0:T, :], lo64[0:T, :],
                                    mybir.AluOpType.add)

            # ---- AllReduce the (32,512) partial across the 8 cores ----
            bin_ = dr.tile([T, K], F32)
            bout = dr.tile([T, K], F32, addr_space="Shared")
            nc.gpsimd.dma_start(bin_[:], part[:])
            nc.gpsimd.collective_compute(
                "AllReduce",
                mybir.AluOpType.add,
                replica_groups=[list(range(NCORES))],
                ins=[bin_.opt()],
                outs=[bout.opt()],
            )
            ofull = sb.tile([T, K], F32)
            nc.sync.dma_start(ofull[:], bout[:])

            # ---- threshold fire ----
            spike = sb.tile([T, K], F32)
            nc.vector.tensor_scalar(spike[:], ofull[:], THRESH, None,
                                    op0=mybir.AluOpType.is_gt)
            pot = sb.tile([T, K], F32)
            nc.vector.tensor_tensor(pot[:], spike[:], ofull[:],
                                    mybir.AluOpType.mult)

            # nspk broadcast to all 32 rows in one matmul: ones(32,32).T @ spike
            nspkb_ps = ps.tile([T, K], F32)
            nc.tensor.matmul(nspkb_ps[:], o32[:], spike[:], start=True, stop=True)
            nrow = sb.tile([1, K], F32)
            nc.vector.tensor_copy(nrow[:], nspkb_ps[0:1, :])

            # onehot(t == clip(32-nspk,0,31)) == (nspk == 32 - t) except the
            # nspk==0 row-31 case, where pot is all-zero anyway.
            # values[k] = sum_t pot * onehot
            pv = sb.tile([T, K], F32)
            nc.vector.scalar_tensor_tensor(pv[:], nspkb_ps[:], tpos[:], pot[:],
                                           op0=mybir.AluOpType.is_equal,
                                           op1=mybir.AluOpType.mult)
            vals_ps = ps.tile([1, K], F32)
            nc.tensor.matmul(vals_ps[:], oc[:], pv[:], start=True, stop=True)

            # v = max(values * (nspk > 0)) * 32
            vm = sb.tile([1, K], F32)
            nc.vector.scalar_tensor_tensor(vm[:], nrow[:], 0.0, vals_ps[:],
                                           op0=mybir.AluOpType.is_gt,
                                           op1=mybir.AluOpType.mult)
            vmax = sb.tile([1, 1], F32)
            nc.vector.tensor_reduce(vmax[:], vm[:], axis=mybir.AxisListType.X,
                                    op=mybir.AluOpType.max)
            vmax32 = sb.tile([1, 1], F32)
            nc.vector.tensor_scalar(vmax32[:], vmax[:], float(T), None,
                                    op0=mybir.AluOpType.mult)

            # total = nspk*values + nspk*vmax32
            t1 = sb.tile([1, K], F32)
            nc.vector.tensor_tensor(t1[:], nrow[:], vals_ps[:],
                                    mybir.AluOpType.mult)
            total = sb.tile([1, K], F32)
            nc.vector.scalar_tensor_tensor(total[:], nrow[:], vmax32[:], t1[:],
                                           op0=mybir.AluOpType.mult,
                                           op1=mybir.AluOpType.add)

            # top-16 nonzero mask: two rounds of (8-max, match-replace-with-0).
            # Zero entries "win" as no-ops and never enter the mask, matching
            # the reference's invalid-winner (-1) behavior.
            work = sb.tile([1, K], F32)
            s8a = sb.tile([1, 8], F32)
            nc.vector.max(s8a[:], total[:])
            nc.vector.match_replace(work[:], s8a[:], total[:], 0.0)
            s8b = sb.tile([1, 8], F32)
            nc.vector.max(s8b[:], work[:])
            nc.vector.match_replace(work[:], s8b[:], work[:], 0.0)

            coef = sb.tile([1, K], F32)  # winner totals, 0 elsewhere
            nc.vector.tensor_tensor(coef[:], total[:], work[:],
                                    mybir.AluOpType.subtract)

            # result = spike * (coef_broadcast > 0)
            coefb_ps = ps.tile([T, K], F32)
            nc.tensor.matmul(coefb_ps[:], orr[:], coef[:], start=True, stop=True)
            res = sb.tile([T, K], F32)
            nc.vector.scalar_tensor_tensor(res[:], coefb_ps[:], 0.0, spike[:],
                                           op0=mybir.AluOpType.is_gt,
                                           op1=mybir.AluOpType.mult)
            nc.sync.dma_start(out_d[:], res[:])

    nc.compile()
    return nc


def _get_nc():
    if "nc" not in _CACHE:
        _CACHE["nc"] = _build_nc()
    return _CACHE["nc"]


def _split_bf16(a):
    """Split fp32 array into (hi, lo) bf16 parts: hi + lo == a to ~2^-18 rel."""
    hi = a.astype(NPBF16)
    lo = (a - hi.astype(np.float32)).astype(NPBF16)
    return hi, lo


def _pack_inputs(rec_field, weight):
    X = np.ascontiguousarray(np.asarray(rec_field, dtype=np.float32).reshape(T, CTOT))
    W = np.ascontiguousarray(np.asarray(weight, dtype=np.float32).reshape(K, CTOT))
    oc = np.ones((T, 1), np.float32)
    o32 = np.ones((T, T), np.float32)
    orr = np.ones((1, T), np.float32)
    tp = (float(T) - np.arange(T, dtype=np.float32)).reshape(T, 1)
    in_maps = []
    for i in range(NCORES):
        xp = np.zeros((T, SHP), np.float32)
        xp[:, :SH] = X[:, i * SH:(i + 1) * SH]
        wp = np.zeros((K, SHP), np.float32)
        wp[:, :SH] = W[:, i * SH:(i + 1) * SH]
        # (contract, n) -> chunks (NCH,128,n)
        xpc = xp.T.reshape(NCH, 128, T)
        wpc = wp.T.reshape(NCH, 128, K)
        xh, xl = _split_bf16(xpc)
        wh, wl = _split_bf16(wpc)
        # per chunk stationary [x_hi | x_lo]: (NCH,128,2T) -> (128, NCH*2T)
        xpk = np.ascontiguousarray(
            np.concatenate([xh, xl], axis=2).transpose(1, 0, 2).reshape(128, NCH * 2 * T))
        # per chunk moving [w_hi | w_lo]: (NCH,128,2K) -> (128, NCH*2K)
        wpk = np.ascontiguousarray(
            np.concatenate([wh, wl], axis=2).transpose(1, 0, 2).reshape(128, NCH * 2 * K))
        in_maps.append({"x": xpk, "w": wpk, "onescol": oc, "ones32": o32,
                        "onesrow": orr, "tpos32": tp})
    return in_maps


def kernel(rec_field, weight, _trace=False, _trace_kwargs=None):
    nc = _get_nc()
    in_maps = _pack_inputs(rec_field, weight)
    r = run_bass_kernel_spmd(nc, in_maps, list(range(NCORES)), trace=_trace,
                             **(_trace_kwargs or {}))
    _CACHE["last_results"] = r
    out = np.asarray(r.results[0]["out"], dtype=np.float32)
    return out.reshape(T, K, 1, 1)


# revision 9
# speedup vs baseline: 1.1251x; 1.1251x over previous
"""Trainium2 Bass kernel for nn_Column (nms_detection).

Computation (matches the reference exactly):
  out[t,k]  = sum_chw rec_field[t,chw] * weight[k,chw]        (32x512 <- contract 100000)
  pot       = out * (out > 10) ; spike = (out > 10)
  nspk[k]   = sum_t spike ; first[k] = min(32 - nspk, 31)
  values[k] = pot[first[k], k] ; v = max_k(values * (nspk>0)) * 32
  total     = nspk*values + nspk*v
  coef      = top-16 nonzero mask of total (== sequential argmax-suppress set)
  result    = spike * coef[broadcast]                          (32x512 of 0.0/1.0)

Distribution: contraction dim (100000) sharded 8 ways (12500 rows/core, padded
to 12544 = 98*128).  Matmul runs in bf16 hi/lo split form (exactly the
decomposition the HW fp32 path uses internally, so fp32-precision): stationary
[x_hi | x_lo] (128,64) per chunk, moving w_hi / w_lo passes accumulating a
(64,N) PSUM folded after the loop.  The K=512 output columns are processed in
two halves so the first half's (32,256) partial AllReduce overlaps the second
half's DMA+matmul; only the second 32KB AllReduce (~10us warm) is exposed.
Every core redundantly computes the k-WTA epilogue; core 0's output is
returned.
"""

import numpy as np
import ml_dtypes

import concourse.bacc as bacc
import concourse.mybir as mybir
from concourse.tile import TileContext
from concourse.bass_utils import run_bass_kernel_spmd

T = 32               # timesteps
K = 512              # out_channels / features
KH = K // 2          # half of the feature columns
CTOT = 100000        # in_channels * rf_size * length (1*50*2000)
NCORES = 8
SH = CTOT // NCORES  # 12500 contraction rows per core
NCH = 98             # 128-row contraction chunks per core
SHP = NCH * 128      # 12544 (zero padded)
GROUP = 7            # chunks per W DMA group (per half: 7*512*128*2B = 896KB)
NG = NCH // GROUP    # 14 groups per half
THRESH = 10.0
F32 = mybir.dt.float32
BF16 = mybir.dt.bfloat16
NPBF16 = ml_dtypes.bfloat16

_CACHE = {}


def _build_nc():
    nc = bacc.Bacc("TRN2", target_bir_lowering=False, debug=False, num_devices=NCORES)

    # x: per chunk c the stationary block [x_hi | x_lo] (128,64) bf16
    x_d = nc.dram_tensor("x", [128, NCH * 2 * T], BF16, kind="ExternalInput")
    # w: half-major; per half h, chunk c: [w_hi (128,256) | w_lo (128,256)]
    w_d = nc.dram_tensor("w", [128, 2 * NCH * K], BF16, kind="ExternalInput")
    oc_d = nc.dram_tensor("onescol", [T, 1], F32, kind="ExternalInput")
    o32_d = nc.dram_tensor("ones32", [T, T], BF16, kind="ExternalInput")
    or_d = nc.dram_tensor("onesrow", [1, T], BF16, kind="ExternalInput")
    tp_d = nc.dram_tensor("tpos32", [T, 1], F32, kind="ExternalInput")
    out_d = nc.dram_tensor("out", [T, K], F32, kind="ExternalOutput")

    with TileContext(nc) as tc:
        with (
            tc.tile_pool(name="sb", bufs=1) as sb,
            tc.tile_pool(name="wp", bufs=4) as wp,
            tc.tile_pool(name="ps", bufs=1, space="PSUM") as ps,
            tc.tile_pool(name="dram", bufs=1, space="DRAM") as dr,
        ):
            # X + consts on the scalar ring first (X is needed before the
            # first matmul); W groups alternate between sync and scalar rings.
            xsb = sb.tile([128, NCH * 2 * T], BF16)
            nc.scalar.dma_start(xsb[:], x_d[:])
            oc = sb.tile([T, 1], F32)
            nc.scalar.dma_start(oc[:], oc_d[:])
            o32 = sb.tile([T, T], BF16)
            nc.scalar.dma_start(o32[:], o32_d[:])
            orr = sb.tile([1, T], BF16)
            nc.scalar.dma_start(orr[:], or_d[:])
            tpos = sb.tile([T, 1], F32)
            nc.scalar.dma_start(tpos[:], tp_d[:])

            accum = [ps.tile([2 * T, KH], F32, name=f"accum{h}") for h in range(2)]
            part = [sb.tile([T, KH], F32, name=f"part{h}") for h in range(2)]
            bins = [dr.tile([T, KH], F32, name=f"bin{h}") for h in range(2)]
            bouts = [dr.tile([T, KH], F32, addr_space="Shared", name=f"bout{h}")
                     for h in range(2)]
            lo64 = sb.tile([2 * T, K], F32)  # staging for the hi/lo folds

            for h in range(2):
                for g in range(NG):
                    wt = wp.tile([128, GROUP * K], BF16, tag="wt")
                    eng = nc.sync if (g % 2 == 0) else nc.scalar
                    base = (h * NG + g) * GROUP * K
                    eng.dma_start(wt[:], w_d[:, base:base + GROUP * K])
                    for c in range(GROUP):
                        cc = g * GROUP + c
                        xst = xsb[:, cc * 2 * T:(cc + 1) * 2 * T]
                        nc.tensor.matmul(
                            accum[h][:],
                            xst,
                            wt[:, c * K:c * K + KH],
                            start=(cc == 0),
                            stop=False,
                        )
                        nc.tensor.matmul(
                            accum[h][:],
                            xst,
                            wt[:, c * K + KH:(c + 1) * K],
                            start=False,
                            stop=(cc == NCH - 1),
                        )

                # fold hi/lo rows: part[h] = accum[h][0:32] + accum[h][32:64].
                # PSUM feeds at most one DVE input and DMA can't read PSUM:
                # copy lo rows to SBUF, move down to partitions 0-31 (gpsimd
                # software DMA -- the HW rings stay on W), then add.
                lo = lo64[:, h * KH:(h + 1) * KH]
                nc.vector.tensor_copy(lo[T:2 * T, :], accum[h][T:2 * T, :])
                nc.gpsimd.dma_start(lo[0:T, :], lo[T:2 * T, :])
                nc.vector.tensor_tensor(part[h][:], accum[h][0:T, :], lo[0:T, :],
                                        mybir.AluOpType.add)
                nc.gpsimd.dma_start(bins[h][:], part[h][:])
                nc.gpsimd.collective_compute(
                    "AllReduce",
                    mybir.AluOpType.add,
                    replica_groups=[list(range(NCORES))],
                    ins=[bins[h].opt()],
                    outs=[bouts[h].opt()],
                )

            # ---- epilogue: half-0 part is hidden under the second AllReduce ----
            ofull = sb.tile([T, K], F32)
            spike = sb.tile([T, K], F32)       # 0/1 as f32 (for final output)
            spikeb = sb.tile([T, K], BF16)     # 0/1 as bf16 (for nspk matmul)
            pot = sb.tile([T, K], F32)
            nspkb_ps = ps.tile([T, K], F32)
            pv = sb.tile([T, K], F32)
            vals_ps = ps.tile([1, K], F32)
            vals = sb.tile([1, K], F32)
            for h in range(2):
                cols = slice(h * KH, (h + 1) * KH)
                nc.sync.dma_start(ofull[:, cols], bouts[h][:])
                nc.vector.tensor_scalar(spike[:, cols], ofull[:, cols], THRESH,
                                        None, op0=mybir.AluOpType.is_gt)
                nc.gpsimd.tensor_scalar(spikeb[:, cols], ofull[:, cols], THRESH,
                                        None, op0=mybir.AluOpType.is_gt)
                # pot = (ofull > 10) * ofull in one fused op
                nc.vector.scalar_tensor_tensor(pot[:, cols], ofull[:, cols],
                                               THRESH, ofull[:, cols],
                                               op0=mybir.AluOpType.is_gt,
                                               op1=mybir.AluOpType.mult)
                # nspk broadcast to all 32 rows: ones(32,32).T @ spikeb (bf16)
                nc.tensor.matmul(nspkb_ps[:, cols], o32[:], spikeb[:, cols],
                                 start=True, stop=True)
                # onehot(nspk == 32-t) * pot ; values[k] = sum_t of that
                nc.vector.scalar_tensor_tensor(pv[:, cols], nspkb_ps[:, cols],
                                               tpos[:], pot[:, cols],
                                               op0=mybir.AluOpType.is_equal,
                                               op1=mybir.AluOpType.mult)
                nc.tensor.matmul(vals_ps[:, cols], oc[:], pv[:, cols],
                                 start=True, stop=True)
                nc.vector.tensor_copy(vals[:, cols], vals_ps[:, cols])

            # v*32 = max(values) * 32  (values[k] is 0 exactly when nspk==0)
            vmax = sb.tile([1, 1], F32)
            nc.vector.tensor_reduce(vmax[:], vals[:], axis=mybir.AxisListType.X,
                                    op=mybir.AluOpType.max)
            vmax32 = sb.tile([1, 1], F32)
            nc.vector.tensor_scalar(vmax32[:], vmax[:], float(T), None,
                                    op0=mybir.AluOpType.mult)
            # total = (values + vmax32) * nspk   (one fused op)
            nrow = sb.tile([1, K], F32)
            nc.scalar.copy(nrow[:], nspkb_ps[0:1, :])
            total = sb.tile([1, K], F32)
            nc.vector.scalar_tensor_tensor(total[:], vals[:], vmax32[:], nrow[:],
                                           op0=mybir.AluOpType.add,
                                           op1=mybir.AluOpType.mult)

            # top-16 nonzero mask: two rounds of (8-max, match-replace-with-0).
            # Zero entries "win" as no-ops and never enter the mask, matching
            # the reference's invalid-winner (-1) behavior.
            work = sb.tile([1, K], F32)
            s8a = sb.tile([1, 8], F32)
            nc.vector.max(s8a[:], total[:])
            nc.vector.match_replace(work[:], s8a[:], total[:], 0.0)
            s8b = sb.tile([1, 8], F32)
            nc.vector.max(s8b[:], work[:])
            nc.vector.match_replace(work[:], s8b[:], work[:], 0.0)

            # winner mask as bf16 0/1 (exact), broadcast via bf16 matmul
            cmask = sb.tile([1, K], BF16)
            nc.vector.tensor_tensor(cmask[:], total[:], work[:],
                                    mybir.AluOpType.is_gt)
            coefb_ps = ps.tile([T, K], F32)
            nc.tensor.matmul(coefb_ps[:], orr[:], cmask[:], start=True, stop=True)
            res = sb.tile([T, K], F32)
            nc.vector.scalar_tensor_tensor(res[:], coefb_ps[:], 0.0, spike[:],
                                           op0=mybir.AluOpType.is_gt,
                                           op1=mybir.AluOpType.mult)
            nc.sync.dma_start(out_d[:], res[:])

    nc.compile()
    return nc


def _get_nc():
    if "nc" not in _CACHE:
        _CACHE["nc"] = _build_nc()
    return _CACHE["nc"]


def _split_bf16(a):
    """Split fp32 array into (hi, lo) bf16 parts: hi + lo == a to ~2^-18 rel."""
    hi = a.astype(NPBF16)
    lo = (a - hi.astype(np.float32)).astype(NPBF16)
    return hi, lo


def _pack_inputs(rec_field, weight):
    X = np.ascontiguousarray(np.asarray(rec_field, dtype=np.float32).reshape(T, CTOT))
    W = np.ascontiguousarray(np.asarray(weight, dtype=np.float32).reshape(K, CTOT))
    oc = np.ones((T, 1), np.float32)
    o32 = np.ones((T, T), NPBF16)
    orr = np.ones((1, T), NPBF16)
    tp = (float(T) - np.arange(T, dtype=np.float32)).reshape(T, 1)
    in_maps = []
    for i in range(NCORES):
        xp = np.zeros((T, SHP), np.float32)
        xp[:, :SH] = X[:, i * SH:(i + 1) * SH]
        wp = np.zeros((K, SHP), np.float32)
        wp[:, :SH] = W[:, i * SH:(i + 1) * SH]
        # (contract, n) -> chunks (NCH,128,n)
        xpc = xp.T.reshape(NCH, 128, T)
        wpc = wp.T.reshape(NCH, 128, K)
        xh, xl = _split_bf16(xpc)
        wh, wl = _split_bf16(wpc)
        # per chunk stationary [x_hi | x_lo]: (NCH,128,2T) -> (128, NCH*2T)
        xpk = np.ascontiguousarray(
            np.concatenate([xh, xl], axis=2).transpose(1, 0, 2).reshape(128, NCH * 2 * T))
        # w: half-major layout; per half h, chunk c: [w_hi[:,h] | w_lo[:,h]]
        # -> (2, NCH, 128, 512) -> (128, 2*NCH*512)
        whl = np.stack([
            np.concatenate([wh[:, :, :KH], wl[:, :, :KH]], axis=2),
            np.concatenate([wh[:, :, KH:], wl[:, :, KH:]], axis=2),
        ])  # (2, NCH, 128, K)
        wpk = np.ascontiguousarray(
            whl.transpose(2, 0, 1, 3).reshape(128, 2 * NCH * K))
        in_maps.append({"x": xpk, "w": wpk, "onescol": oc, "ones32": o32,
                        "onesrow": orr, "tpos32": tp})
    return in_maps


def kernel(rec_field, weight, _trace=False, _trace_kwargs=None):
    nc = _get_nc()
    in_maps = _pack_inputs(rec_field, weight)
    r = run_bass_kernel_spmd(nc, in_maps, list(range(NCORES)), trace=_trace,
                             **(_trace_kwargs or {}))
    _CACHE["last_results"] = r
    out = np.asarray(r.results[0]["out"], dtype=np.float32)
    return out.reshape(T, K, 1, 1)
